# revision 30
# baseline (speedup 1.0000x reference)
"""Trainium2 Bass kernel: causal self-attention (GQA + RoPE) for
B=1, T=2048, C=2048, H=16 query heads, HKV=4 KV heads, D=128.

Sharding: tensor-parallel over heads across 8 NeuronCores. Core m computes
query heads {2m, 2m+1} and the single KV head (m//2) those heads attend to,
plus the o_proj partial product for its 256 input columns. The host sums the
8 partial outputs (the TP all-reduce).

All matmul operands are bf16 (measured end-to-end emulation error 3.6e-3
against the fp64 reference, vs the 2e-2 gate); PSUM accumulation stays fp32.
bf16 halves HBM traffic (x alone is 8.4MB/core), halves DVE element cost,
and halves LDWEIGHTS time vs fp32r. The v-transpose chain stays fp32
(PE matmul-transpose; bf16 DMA-transpose would xbar-serialize the queues).

Schedule (program order = PE order). H(tq) = scores/softmax/AV for one query
block, O(tq) = its o_proj. O(tq) is deferred one slot so the DVE ymul that
produces ys always completes during unrelated PE work, and O's inputs are
never waited on:

  warmup MMs (beats the HAM clock-gate: PE busy from ~7us so the 2.4GHz
  un-throttle lands right as the first real matmul issues)
  pair0 [t0,t1]  vt0 H0 vt1 H1 O0  pair1 [t2,t3]  O1 vt2 H2 vt3 H3 O2 O3

Each projection half holds only 4 PSUM banks (one x chunk [128,1024] feeds
both halves back-to-back), so one unified PSUM pool (acc=4, sp=2, nrm=2
banks) serves the whole kernel with no pool-transition barriers.

DMA plan: first-use-ordered weight chunks then wo on the gpsimd SWDGE queue
(~2MB before the x stream peaks); x chunks + per-tile cos/sin slices + rope
partition-swaps on the sync queue; output rides gpsimd early, both queues
for the last two blocks to halve the tail drain. Engine plan: ACT does exp,
v-eviction and a quarter of o-evictions; DVE does q/k evictions, RoPE,
normalization and the rest; GpSimd does the causal-mask multiplies (SBUF
only) so they never queue behind the exp/eviction streams.

Attention per (tq, h): ST = k'T.T @ q'T per 128-key tile (causally skipped
column prefixes), P = exp(ST) on ACT (bf16 out, no max subtraction: logits
are bounded), diagonal masked by a triangular multiply, column sums via
ones-matmul + AV matmul accumulated in PSUM, y normalized by
reciprocal(sums), o_proj partials evicted bf16 and DMA'd.
"""

import math
import numpy as np
from contextlib import ExitStack

import ml_dtypes

import concourse.bass as bass
import concourse.bacc as bacc
import concourse.tile as tile
from concourse import mybir
from concourse.bass_utils import run_bass_kernel_spmd
from concourse.masks import make_identity

B, T, C = 1, 2048, 2048
H, HKV = 16, 4
D = 128
NCORES = 8
HL = H // NCORES          # query heads per core
TQ = 512                  # query tile width (one fp32 PSUM bank)
NT = T // TQ              # 4 query tiles
NK = T // D               # 16 key tiles
NCT = C // 128            # 16 contraction tiles over the model dim
F32 = mybir.dt.float32
BF16 = mybir.dt.bfloat16
Id = mybir.ActivationFunctionType.Identity
Exp = mybir.ActivationFunctionType.Exp

_CACHE: dict = {}


def _build():
    nc = bacc.Bacc(None, target_bir_lowering=False, debug=False)
    scale = 1.0 / math.sqrt(D)
    with tile.TileContext(nc) as tc, ExitStack() as ctx:
        dram = ctx.enter_context(tc.tile_pool(name="dram", bufs=1, space="DRAM"))

        def din(name, shape, dt=BF16):
            return dram.tile(shape, dt, kind="ExternalInput", name=name,
                             uniquify=False)

        xt_d = din("xt", [C, T])          # x[0].T
        # weights host-pre-tiled to SBUF layout [128, ...]; chunk c of wq at
        # cols c*HL*D.., of wk/wv at c*D..
        wq_d = din("wq", [128, NCT * HL * D])
        wk_d = din("wk", [128, NCT * D])
        wv_d = din("wv", [128, NCT * D])
        wo_d = din("wo", [128, HL * C])
        bcon_d = din("bcon", [D, 4], F32)   # [bq0 bq1 bk bv], bq pre-scaled
        mcon_d = din("mcon", [D, 4 * D])    # [tri | tri2(2D) | ones]
        cos_d = din("cost", [D, T])         # cos[0].T
        sin_d = din("sins", [D, T])         # sin[0].T with rows 0:64 negated
        out_d = dram.tile([T, C], BF16, kind="ExternalOutput",
                          name="out", uniquify=False)

        const = ctx.enter_context(tc.tile_pool(name="const", bufs=1))
        wq_s = const.tile([128, NCT * HL * D], BF16, name="wq_s")
        wk_s = const.tile([128, NCT * D], BF16, name="wk_s")
        wv_s = const.tile([128, NCT * D], BF16, name="wv_s")
        wo_s = const.tile([128, HL * C], BF16, name="wo_s")
        bcon_s = const.tile([128, 4], F32, name="bcon_s")
        mcon_s = const.tile([128, 4 * D], BF16, name="mcon_s")
        cos_s = const.tile([128, T], BF16, name="cos_s")
        sin_s = const.tile([128, T], BF16, name="sin_s")
        ident_s = const.tile([128, 128], F32, name="ident_s")
        warm_s = const.tile([128, 128], F32, name="warm_s")
        xpool = ctx.enter_context(tc.tile_pool(name="xpool", bufs=NCT))
        prepool = ctx.enter_context(tc.tile_pool(name="prepool", bufs=4))
        rpool = ctx.enter_context(tc.tile_pool(name="rpool", bufs=3))
        ppool = ctx.enter_context(tc.tile_pool(name="ppool", bufs=8))
        opool = ctx.enter_context(tc.tile_pool(name="opool", bufs=3))
        # single PSUM pool for the whole kernel: acc(4) + sp(2) + nrm(2) = 8
        # banks; no pool-transition barriers between phases.
        psum = ctx.enter_context(tc.tile_pool(name="psum", bufs=1,
                                              space="PSUM"))

        # identity first: it runs on the GpSimd ENGINE ahead of the weight
        # dma_start issues, so the PE warmup below can begin at ~7us.
        make_identity(nc, ident_s[:])
        # first four x chunks ride the gpsimd queue AHEAD of the weights:
        # the first c-loop iterations get their data while the sync queue
        # works through its own x stream, smoothing the DMA-paced ramp
        x_head = []
        for c in range(4):
            xt = xpool.tile([128, 2 * TQ], BF16, tag="x")
            nc.gpsimd.dma_start(out=xt[:], in_=xt_d[bass.ts(c, 128), 0:2 * TQ])
            x_head.append(xt)
        # Weight loads on the GpSimd SWDGE queue in first-use order (proj
        # matmuls per c go v,k,q0,q1), chunked so the c=0 matmuls wait on
        # one ~128KB chunk rather than the full set. wo follows immediately
        # (first needed by O0 at ~1/4 of the kernel).
        WCH = 4                           # c-chunks per weight DMA
        for g in range(NCT // WCH):
            qsl = bass.ts(g, WCH * HL * D)
            ksl = bass.ts(g, WCH * D)
            nc.gpsimd.dma_start(out=wv_s[:, ksl], in_=wv_d[:, ksl])
            nc.gpsimd.dma_start(out=wk_s[:, ksl], in_=wk_d[:, ksl])
            nc.gpsimd.dma_start(out=wq_s[:, qsl], in_=wq_d[:, qsl])
        for g in range(2):
            osl = bass.ts(g, C)
            nc.gpsimd.dma_start(out=wo_s[:, osl], in_=wo_d[:, osl])

        bq0, bq1 = bcon_s[:, 0:1], bcon_s[:, 1:2]
        bk_b, bv_b = bcon_s[:, 2:3], bcon_s[:, 3:4]
        tri_s = mcon_s[:, 0:D]
        tri2_s = mcon_s[:, D:3 * D]
        ones_s = mcon_s[:, 3 * D:4 * D]

        act = ctx.enter_context(tc.tile_pool(name="act", bufs=1))
        qr = [act.tile([128, T], BF16, name=f"qr{h}_s") for h in range(HL)]
        kr_s = act.tile([128, T], BF16, name="kr_s")
        vT_s = act.tile([128, T], F32, name="vT_s")
        v_s = act.tile([128, NK * D], BF16, name="v_s")
        ys = [act.tile([128, T], BF16, name=f"y{h}_s") for h in range(HL)]


        # PE warmup: ~3.4us of accumulating matmuls on the identity so the
        # HAM clock-gate opens to 2.4GHz right as the first x chunk lands.
        # (Accumulation group so DCE can't drop the intermediate writes.)
        warm = psum.tile([128, 128], F32, tag="acc", bufs=4, name="warm")
        NWARM = 12
        for i in range(NWARM):
            nc.tensor.matmul(warm[:], ident_s[:], ident_s[:],
                             start=(i == 0), stop=(i == NWARM - 1))
        nc.scalar.copy(warm_s[:], warm[:])

        def rope(dst, pre, t):
            """dst = pre*cos + rot_half(pre)*sin on columns [t*TQ,(t+1)*TQ).

            All bf16 on DVE (2x 16-bit mode). Partition-swap copies ride the
            sync queue behind this pair's x chunks.
            """
            sl = bass.ts(t, TQ)
            rot = rpool.tile([128, TQ], BF16, tag="rot")
            nc.sync.dma_start(out=rot[0:64, :], in_=pre[64:128, :])
            nc.sync.dma_start(out=rot[64:128, :], in_=pre[0:64, :])
            nc.vector.tensor_mul(rot[:], rot[:], sin_s[:, sl])
            tmp = rpool.tile([128, TQ], BF16, tag="rtmp")
            nc.vector.tensor_mul(tmp[:], pre[:], cos_s[:, sl])
            nc.vector.tensor_add(dst, tmp[:], rot[:])

        def proj_pair(tp):
            # One x chunk [128, 1024] per c feeds both halves (one dma_start
            # each: the ~0.65us per-issue sequencer cost makes finer chunks
            # a net loss). Each half keeps its own 4-bank accumulator set
            # processed back-to-back so proj never holds more than 4 PSUM
            # banks (attn tags keep theirs).
            xts = list(x_head) if tp == 0 else []
            for c in range(len(xts), NCT):
                xt = xpool.tile([128, 2 * TQ], BF16, tag="x")
                nc.sync.dma_start(
                    out=xt[:],
                    in_=xt_d[bass.ts(c, 128), tp * 2 * TQ:(tp + 1) * 2 * TQ])
                xts.append(xt)
            # cos/sin slices for this pair's two tiles, behind the x chunks
            for t in (2 * tp, 2 * tp + 1):
                sl = bass.ts(t, TQ)
                nc.sync.dma_start(out=cos_s[:, sl], in_=cos_d[:, sl])
                nc.sync.dma_start(out=sin_s[:, sl], in_=sin_d[:, sl])
            if tp == 0:
                # biases/masks behind pair0's x so the x stream (which paces
                # the first c-loop) owns the early bandwidth; these are only
                # needed at eviction (~24us) and H0 (~32us).
                nc.sync.dma_start(out=bcon_s[:], in_=bcon_d[:])
                nc.sync.dma_start(out=mcon_s[:], in_=mcon_d[:])
            for half in range(2):
                t = 2 * tp + half
                sl = bass.ts(t, TQ)
                # issue order (v,k,q0,q1) matches eviction order below so the
                # next half's first MM waits only on the first eviction
                accs = [psum.tile([128, TQ], F32, tag="acc", bufs=4,
                                  name=f"acc{j}") for j in range(4)]
                for c in range(NCT):
                    xh = xts[c][:, bass.ts(half, TQ)]
                    st, sp = (c == 0), (c == NCT - 1)
                    base = c * HL * D
                    nc.tensor.matmul(accs[0][:], wv_s[:, bass.ts(c, D)],
                                     xh, start=st, stop=sp)
                    nc.tensor.matmul(accs[1][:], wk_s[:, bass.ts(c, D)],
                                     xh, start=st, stop=sp)
                    nc.tensor.matmul(accs[2][:], wq_s[:, base:base + D],
                                     xh, start=st, stop=sp)
                    nc.tensor.matmul(accs[3][:],
                                     wq_s[:, base + D:base + 2 * D],
                                     xh, start=st, stop=sp)
                # ACT keeps only the v eviction (vtrans needs it); q/k go to
                # DVE so exp never queues behind them at H starts.
                nc.scalar.activation(vT_s[:, sl], accs[0][:], Id,
                                     bias=bv_b)
                prek = prepool.tile([128, TQ], BF16, tag="prek")
                nc.vector.tensor_scalar(prek[:], accs[1][:], 1.0,
                                        bk_b, mybir.AluOpType.mult,
                                        mybir.AluOpType.add)
                pre0 = prepool.tile([128, TQ], BF16, tag="pre0")
                pre1 = prepool.tile([128, TQ], BF16, tag="pre1")
                nc.vector.tensor_scalar(pre0[:], accs[2][:], scale,
                                        bq0, mybir.AluOpType.mult,
                                        mybir.AluOpType.add)
                nc.vector.tensor_scalar(pre1[:], accs[3][:], scale,
                                        bq1, mybir.AluOpType.mult,
                                        mybir.AluOpType.add)
                rope(kr_s[:, sl], prek, t)
                rope(qr[0][:, sl], pre0, t)
                rope(qr[1][:, sl], pre1, t)

        def vtrans(t):
            # transpose t's 4 new v tiles to natural layout (fp32 PE path),
            # evict bf16 on ACT
            for tk in range(4 * t, 4 * t + 4):
                vtp = psum.tile([128, D], F32, tag="acc", bufs=4, name="vtp")
                nc.tensor.transpose(vtp[:], vT_s[:, bass.ts(tk, D)],
                                    ident_s[:])
                nc.vector.tensor_copy(v_s[:, bass.ts(tk, D)], vtp[:])

        def attn_heads(tq):
            ntk = 4 * tq + 4
            # Diagonal tiles first: their exp->mask chains complete while the
            # (mask-free) off-diagonal tiles stream, so the masked 128-column
            # matmuls (emitted after two off-diagonal consumes) never stall
            # the PE. For tq=0 every tile is diagonal; keep the plain order
            # with inline masked consumes there.
            if tq == 0:
                order = list(range(4))
            else:
                # one off-diagonal tile first: its full-width consume
                # initializes every PSUM column (start=True), so all later
                # partial-width writes accumulate
                order = ([0] + list(range(4 * tq, ntk))
                         + list(range(1, 4 * tq)))
            for h in range(HL):
                sump = psum.tile([128, TQ], F32, tag="nrm", bufs=2,
                                 name="sump")
                yp = psum.tile([128, TQ], F32, tag="nrm", bufs=2, name="yp")

                def score_exp(tk):
                    r = max(tk * D - tq * TQ, 0)  # masked col prefix
                    rr = min(r, TQ - 2 * D)       # keep free dim >= 256
                    sp_ = psum.tile([128, TQ], F32, tag="sp", bufs=2,
                                    name="sp")
                    nc.tensor.matmul(
                        sp_[:, rr:], kr_s[:, bass.ts(tk, D)],
                        qr[h][:, tq * TQ + rr:(tq + 1) * TQ],
                        start=True, stop=True)
                    pt = ppool.tile([128, TQ], BF16, tag="p")
                    nc.scalar.activation(pt[:, rr:], sp_[:, rr:], Exp)
                    if tk * D >= tq * TQ:  # diagonal: causal mask
                        # DVE is idle during H0/H1 (o-evictions only start
                        # with O0) and ~3x faster than GpSimd on 2-input
                        # ops; H2/H3 keep GpSimd so masks never queue
                        # behind the previous O block's DVE evictions
                        meng = nc.vector if tq <= 1 else nc.gpsimd
                        if r > rr:
                            meng.tensor_mul(pt[:, rr:r + D],
                                            pt[:, rr:r + D], tri2_s)
                        else:
                            meng.tensor_mul(pt[:, r:r + D],
                                            pt[:, r:r + D], tri_s)
                    return pt, r, rr

                def consume(pt, lo, hi, tkv, first=False, last=False):
                    # exactly one start=True per accumulation group: it
                    # clears has_written for the whole bank, and its write
                    # must cover all columns later writes accumulate into
                    nc.tensor.matmul(sump[:, lo:hi], ones_s, pt[:, lo:hi],
                                     start=first, stop=last)
                    nc.tensor.matmul(yp[:, lo:hi], v_s[:, bass.ts(tkv, D)],
                                     pt[:, lo:hi], start=first, stop=last)

                pts = {}
                deferred = []
                ndone = 0
                for i in range(len(order) + 1):
                    if i < len(order):
                        pts[order[i]] = score_exp(order[i])
                    if i >= 1:
                        tk = order[i - 1]
                        pt, r, rr = pts.pop(tk)
                        if tq == 0:
                            # tile 0 consumed full-width inline (only its
                            # mask - the first ready - gates the PE); later
                            # tiles defer their masked columns to the end
                            if tk == 0:
                                consume(pt, 0, TQ, tk, first=True)
                                continue
                            deferred.append((pt, r, tk))
                            if r + D < TQ:
                                consume(pt, r + D, TQ, tk)
                            if tk == ntk - 1:
                                for j, (pt_, r_, tk_) in enumerate(deferred):
                                    consume(pt_, r_, r_ + D, tk_,
                                            last=(j == len(deferred) - 1))
                                deferred = []
                        elif tk * D >= tq * TQ:  # diagonal
                            # clean columns now; masked ones deferred past
                            # the exp->mask chain
                            deferred.append((pt, r, tk))
                            if r + D < TQ:
                                consume(pt, r + D, TQ, tk)
                        else:
                            ndone += 1
                            consume(pt, 0, TQ, tk, first=(ndone == 1),
                                    last=(ndone == 4 * tq))
                            if ndone == 3:
                                # flush after the 3rd off-diagonal consume:
                                # enough slack for the diagonal exp->mask
                                # chains without starving the pt pool
                                for pt_, r_, tk_ in deferred:
                                    consume(pt_, r_, r_ + D, tk_)
                                deferred = []
                rec = rpool.tile([128, TQ], F32, tag="rec")
                nc.vector.reciprocal_approx_fast(rec[:], sump[:])
                nc.vector.tensor_mul(ys[h][:, bass.ts(tq, TQ)], yp[:],
                                     rec[:])

        def o_proj(tq):
            # o_proj for the 4 row-tiles of block tq; evict the 4 column
            # tiles into one wide bf16 tile -> single DMA with 4KB lines.
            # Runs one slot after attn_heads(tq) so every input is ready.
            for tt in range(4):
                t = 4 * tq + tt
                wide = opool.tile([128, C], BF16, tag="oev")
                last_tile = tq == 3 and tt == 3
                for n in range(NT):
                    op_ = psum.tile([128, TQ], F32, tag="acc", bufs=4,
                                    name="op")
                    for h in range(HL):
                        nc.tensor.matmul(
                            op_[:], ys[h][:, bass.ts(t, D)],
                            wo_s[:, h * C + n * TQ:h * C + (n + 1) * TQ],
                            start=(h == 0), stop=(h == HL - 1))
                    if n == 3:
                        nc.scalar.copy(wide[:, bass.ts(n, TQ)], op_[:])
                    else:
                        nc.vector.tensor_copy(wide[:, bass.ts(n, TQ)], op_[:])
                    if last_tile:
                        # drain the final row-tile per column block on
                        # alternating queues so the tail is ~1 transfer deep
                        eng = nc.sync if n % 2 else nc.gpsimd
                        eng.dma_start(
                            out=out_d[bass.ts(t, D), n * TQ:(n + 1) * TQ],
                            in_=wide[:, bass.ts(n, TQ)])
                if not last_tile:
                    if tq >= 2 and tt % 2 == 1:
                        nc.sync.dma_start(out=out_d[bass.ts(t, D), :],
                                          in_=wide[:])
                    else:
                        nc.gpsimd.dma_start(out=out_d[bass.ts(t, D), :],
                                            in_=wide[:])

        proj_pair(0)
        vtrans(0)
        attn_heads(0)
        vtrans(1)
        attn_heads(1)
        o_proj(0)
        proj_pair(1)
        o_proj(1)
        vtrans(2)
        attn_heads(2)
        vtrans(3)
        attn_heads(3)
        o_proj(2)
        o_proj(3)
    nc.compile()
    return nc


def _get_nc():
    if "nc" not in _CACHE:
        _CACHE["nc"] = _build()
    return _CACHE["nc"]


def _prep_inputs(x, cos, sin, Wq, bq, Wk, bk, Wv, bv, Wo):
    f = np.float32
    bf = ml_dtypes.bfloat16
    xT = np.ascontiguousarray(x[0].T, dtype=bf)
    cosT = np.ascontiguousarray(cos[0].T, dtype=bf)
    sinT = np.asarray(sin[0].T, dtype=f)
    sins = np.concatenate([-sinT[:64], sinT[64:]], axis=0)
    sins = np.ascontiguousarray(sins, dtype=bf)
    idx = np.arange(D)
    tri = (idx[:, None] <= idx[None, :]).astype(bf)
    mcon = np.concatenate(
        [tri, np.zeros((D, D), bf), tri, np.ones((D, D), bf)], axis=1)
    mcon = np.ascontiguousarray(mcon)
    scale = np.float32(1.0 / math.sqrt(D))
    in_maps = []

    def ptile(a):
        """[K*128, N] -> partition-major [128, K*N] matching the SBUF tiles."""
        k = a.shape[0] // 128
        return np.ascontiguousarray(
            a.reshape(k, 128, a.shape[1]).transpose(1, 0, 2).reshape(128, -1)
            .astype(bf))

    for m in range(NCORES):
        g = m // 2
        wq_m = ptile(Wq[m * 256:(m + 1) * 256, :].T.astype(f))
        wk_m = ptile(Wk[g * 128:(g + 1) * 128, :].T.astype(f))
        wv_m = ptile(Wv[g * 128:(g + 1) * 128, :].T.astype(f))
        wo_m = ptile(Wo[:, m * 256:(m + 1) * 256].T.astype(f))
        bq_m = (bq[m * 256:(m + 1) * 256] * scale).reshape(HL, D).T
        bcon = np.stack([bq_m[:, 0], bq_m[:, 1],
                         bk[g * 128:(g + 1) * 128],
                         bv[g * 128:(g + 1) * 128]], axis=1)
        bcon = np.ascontiguousarray(bcon, dtype=f)
        in_maps.append({
            "xt": xT, "wq": wq_m, "wk": wk_m, "wv": wv_m, "wo": wo_m,
            "bcon": bcon, "mcon": mcon, "cost": cosT, "sins": sins,
        })
    return in_maps


def kernel(x, cos, sin, Wq, bq, Wk, bk, Wv, bv, Wo, _trace=False):
    x, cos, sin = np.asarray(x), np.asarray(cos), np.asarray(sin)
    Wq, bq = np.asarray(Wq), np.asarray(bq)
    Wk, bk = np.asarray(Wk), np.asarray(bk)
    Wv, bv = np.asarray(Wv), np.asarray(bv)
    Wo = np.asarray(Wo)
    nc = _get_nc()
    in_maps = _prep_inputs(x, cos, sin, Wq, bq, Wk, bk, Wv, bv, Wo)
    res = run_bass_kernel_spmd(nc, in_maps, core_ids=list(range(NCORES)),
                               trace=_trace)
    out = res.results[0]["out"].astype(np.float64)
    for m in range(1, NCORES):
        out += res.results[m]["out"]
    out = out.astype(np.float32).reshape(B, T, C)
    if _trace:
        _CACHE["last_result"] = res
    return out


# revision 31
# speedup vs baseline: 1.0044x; 1.0044x over previous
"""Trainium2 Bass kernel: causal self-attention (GQA + RoPE) for
B=1, T=2048, C=2048, H=16 query heads, HKV=4 KV heads, D=128.

Sharding: tensor-parallel over heads across 8 NeuronCores. Core m computes
query heads {2m, 2m+1} and the single KV head (m//2) those heads attend to,
plus the o_proj partial product for its 256 input columns. The host sums the
8 partial outputs (the TP all-reduce).

All matmul operands are bf16 (measured end-to-end emulation error 3.6e-3
against the fp64 reference, vs the 2e-2 gate); PSUM accumulation stays fp32.
bf16 halves HBM traffic (x alone is 8.4MB/core), halves DVE element cost,
and halves LDWEIGHTS time vs fp32r. The v-transpose chain stays fp32
(PE matmul-transpose; bf16 DMA-transpose would xbar-serialize the queues).

Schedule (program order = PE order). H(tq) = scores/softmax/AV for one query
block, O(tq) = its o_proj. O(tq) is deferred one slot so the DVE ymul that
produces ys always completes during unrelated PE work, and O's inputs are
never waited on:

  warmup MMs (beats the HAM clock-gate: PE busy from ~7us so the 2.4GHz
  un-throttle lands right as the first real matmul issues)
  pair0 [t0,t1]  vt0 H0 vt1 H1 O0  pair1 [t2,t3]  O1 vt2 H2 vt3 H3 O2 O3

Each projection half holds only 4 PSUM banks (one x chunk [128,1024] feeds
both halves back-to-back), so one unified PSUM pool (acc=4, sp=2, nrm=2
banks) serves the whole kernel with no pool-transition barriers.

DMA plan: first-use-ordered weight chunks then wo on the gpsimd SWDGE queue
(~2MB before the x stream peaks); x chunks + per-tile cos/sin slices + rope
partition-swaps on the sync queue; output rides gpsimd early, both queues
for the last two blocks to halve the tail drain. Engine plan: ACT does exp,
v-eviction and a quarter of o-evictions; DVE does q/k evictions, RoPE,
normalization and the rest; GpSimd does the causal-mask multiplies (SBUF
only) so they never queue behind the exp/eviction streams.

Attention per (tq, h): ST = k'T.T @ q'T per 128-key tile (causally skipped
column prefixes), P = exp(ST) on ACT (bf16 out, no max subtraction: logits
are bounded), diagonal masked by a triangular multiply, column sums via
ones-matmul + AV matmul accumulated in PSUM, y normalized by
reciprocal(sums), o_proj partials evicted bf16 and DMA'd.
"""

import math
import numpy as np
from contextlib import ExitStack

import ml_dtypes

import concourse.bass as bass
import concourse.bacc as bacc
import concourse.tile as tile
from concourse import mybir
from concourse.bass_utils import run_bass_kernel_spmd
from concourse.masks import make_identity

B, T, C = 1, 2048, 2048
H, HKV = 16, 4
D = 128
NCORES = 8
HL = H // NCORES          # query heads per core
TQ = 512                  # query tile width (one fp32 PSUM bank)
NT = T // TQ              # 4 query tiles
NK = T // D               # 16 key tiles
NCT = C // 128            # 16 contraction tiles over the model dim
F32 = mybir.dt.float32
BF16 = mybir.dt.bfloat16
Id = mybir.ActivationFunctionType.Identity
Exp = mybir.ActivationFunctionType.Exp

_CACHE: dict = {}


def _build():
    nc = bacc.Bacc(None, target_bir_lowering=False, debug=False)
    scale = 1.0 / math.sqrt(D)
    with tile.TileContext(nc) as tc, ExitStack() as ctx:
        dram = ctx.enter_context(tc.tile_pool(name="dram", bufs=1, space="DRAM"))

        def din(name, shape, dt=BF16):
            return dram.tile(shape, dt, kind="ExternalInput", name=name,
                             uniquify=False)

        xt_d = din("xt", [C, T])          # x[0].T
        # weights host-pre-tiled to SBUF layout [128, ...]; chunk c of wq at
        # cols c*HL*D.., of wk/wv at c*D..
        wq_d = din("wq", [128, NCT * HL * D])
        wk_d = din("wk", [128, NCT * D])
        wv_d = din("wv", [128, NCT * D])
        wo_d = din("wo", [128, HL * C])
        bcon_d = din("bcon", [D, 4], F32)   # [bq0 bq1 bk bv], bq pre-scaled
        mcon_d = din("mcon", [D, 4 * D])    # [tri | tri2(2D) | ones]
        cos_d = din("cost", [D, T])         # cos[0].T
        sin_d = din("sins", [D, T])         # sin[0].T with rows 0:64 negated
        out_d = dram.tile([T, C], BF16, kind="ExternalOutput",
                          name="out", uniquify=False)

        const = ctx.enter_context(tc.tile_pool(name="const", bufs=1))
        wq_s = const.tile([128, NCT * HL * D], BF16, name="wq_s")
        wk_s = const.tile([128, NCT * D], BF16, name="wk_s")
        wv_s = const.tile([128, NCT * D], BF16, name="wv_s")
        wo_s = const.tile([128, HL * C], BF16, name="wo_s")
        bcon_s = const.tile([128, 4], F32, name="bcon_s")
        mcon_s = const.tile([128, 4 * D], BF16, name="mcon_s")
        cos_s = const.tile([128, T], BF16, name="cos_s")
        sin_s = const.tile([128, T], BF16, name="sin_s")
        ident_s = const.tile([128, 128], F32, name="ident_s")
        warm_s = const.tile([128, 128], F32, name="warm_s")
        # identity first: it runs on the GpSimd ENGINE ahead of the weight
        # dma_start issues, so the PE warmup below can begin at ~7us.
        make_identity(nc, ident_s[:])
        # Weight loads on the GpSimd SWDGE queue in first-use order (proj
        # matmuls per c go v,k,q0,q1), chunked so the c=0 matmuls wait on
        # one ~128KB chunk rather than the full set. wo follows immediately
        # (first needed by O0 at ~1/4 of the kernel).
        WCH = 4                           # c-chunks per weight DMA
        for g in range(NCT // WCH):
            qsl = bass.ts(g, WCH * HL * D)
            ksl = bass.ts(g, WCH * D)
            nc.gpsimd.dma_start(out=wv_s[:, ksl], in_=wv_d[:, ksl])
            nc.gpsimd.dma_start(out=wk_s[:, ksl], in_=wk_d[:, ksl])
            nc.gpsimd.dma_start(out=wq_s[:, qsl], in_=wq_d[:, qsl])
        for g in range(2):
            osl = bass.ts(g, C)
            nc.gpsimd.dma_start(out=wo_s[:, osl], in_=wo_d[:, osl])

        bq0, bq1 = bcon_s[:, 0:1], bcon_s[:, 1:2]
        bk_b, bv_b = bcon_s[:, 2:3], bcon_s[:, 3:4]
        tri_s = mcon_s[:, 0:D]
        tri2_s = mcon_s[:, D:3 * D]
        ones_s = mcon_s[:, 3 * D:4 * D]

        act = ctx.enter_context(tc.tile_pool(name="act", bufs=1))
        qr = [act.tile([128, T], BF16, name=f"qr{h}_s") for h in range(HL)]
        kr_s = act.tile([128, T], BF16, name="kr_s")
        vT_s = act.tile([128, T], F32, name="vT_s")
        v_s = act.tile([128, NK * D], BF16, name="v_s")
        ys = [act.tile([128, T], BF16, name=f"y{h}_s") for h in range(HL)]

        xpool = ctx.enter_context(tc.tile_pool(name="xpool", bufs=NCT))
        prepool = ctx.enter_context(tc.tile_pool(name="prepool", bufs=4))
        rpool = ctx.enter_context(tc.tile_pool(name="rpool", bufs=3))
        ppool = ctx.enter_context(tc.tile_pool(name="ppool", bufs=8))
        opool = ctx.enter_context(tc.tile_pool(name="opool", bufs=3))
        # single PSUM pool for the whole kernel: acc(4) + sp(2) + nrm(2) = 8
        # banks; no pool-transition barriers between phases.
        psum = ctx.enter_context(tc.tile_pool(name="psum", bufs=1,
                                              space="PSUM"))

        # PE warmup: ~3.4us of accumulating matmuls on the identity so the
        # HAM clock-gate opens to 2.4GHz right as the first x chunk lands.
        # (Accumulation group so DCE can't drop the intermediate writes.)
        warm = psum.tile([128, 128], F32, tag="acc", bufs=4, name="warm")
        NWARM = 12
        for i in range(NWARM):
            nc.tensor.matmul(warm[:], ident_s[:], ident_s[:],
                             start=(i == 0), stop=(i == NWARM - 1))
        nc.scalar.copy(warm_s[:], warm[:])

        def rope(dst, pre, t):
            """dst = pre*cos + rot_half(pre)*sin on columns [t*TQ,(t+1)*TQ).

            All bf16 on DVE (2x 16-bit mode). Partition-swap copies ride the
            sync queue behind this pair's x chunks.
            """
            sl = bass.ts(t, TQ)
            rot = rpool.tile([128, TQ], BF16, tag="rot")
            nc.sync.dma_start(out=rot[0:64, :], in_=pre[64:128, :])
            nc.sync.dma_start(out=rot[64:128, :], in_=pre[0:64, :])
            nc.vector.tensor_mul(rot[:], rot[:], sin_s[:, sl])
            tmp = rpool.tile([128, TQ], BF16, tag="rtmp")
            nc.vector.tensor_mul(tmp[:], pre[:], cos_s[:, sl])
            nc.vector.tensor_add(dst, tmp[:], rot[:])

        def proj_pair(tp):
            # One x chunk [128, 1024] per c feeds both halves (one dma_start
            # each: the ~0.65us per-issue sequencer cost makes finer chunks
            # a net loss). Each half keeps its own 4-bank accumulator set
            # processed back-to-back so proj never holds more than 4 PSUM
            # banks (attn tags keep theirs).
            xts = []
            for c in range(NCT):
                xt = xpool.tile([128, 2 * TQ], BF16, tag="x")
                nc.sync.dma_start(
                    out=xt[:],
                    in_=xt_d[bass.ts(c, 128), tp * 2 * TQ:(tp + 1) * 2 * TQ])
                xts.append(xt)
            # cos/sin slices for this pair's two tiles, behind the x chunks
            for t in (2 * tp, 2 * tp + 1):
                sl = bass.ts(t, TQ)
                nc.sync.dma_start(out=cos_s[:, sl], in_=cos_d[:, sl])
                nc.sync.dma_start(out=sin_s[:, sl], in_=sin_d[:, sl])
            if tp == 0:
                # biases/masks behind pair0's x so the x stream (which paces
                # the first c-loop) owns the early bandwidth; these are only
                # needed at eviction (~24us) and H0 (~32us).
                nc.sync.dma_start(out=bcon_s[:], in_=bcon_d[:])
                nc.sync.dma_start(out=mcon_s[:], in_=mcon_d[:])
            for half in range(2):
                t = 2 * tp + half
                sl = bass.ts(t, TQ)
                # issue order (v,k,q0,q1) matches eviction order below so the
                # next half's first MM waits only on the first eviction
                accs = [psum.tile([128, TQ], F32, tag="acc", bufs=4,
                                  name=f"acc{j}") for j in range(4)]
                for c in range(NCT):
                    xh = xts[c][:, bass.ts(half, TQ)]
                    st, sp = (c == 0), (c == NCT - 1)
                    base = c * HL * D
                    nc.tensor.matmul(accs[0][:], wv_s[:, bass.ts(c, D)],
                                     xh, start=st, stop=sp)
                    nc.tensor.matmul(accs[1][:], wk_s[:, bass.ts(c, D)],
                                     xh, start=st, stop=sp)
                    nc.tensor.matmul(accs[2][:], wq_s[:, base:base + D],
                                     xh, start=st, stop=sp)
                    nc.tensor.matmul(accs[3][:],
                                     wq_s[:, base + D:base + 2 * D],
                                     xh, start=st, stop=sp)
                # ACT keeps only the v eviction (vtrans needs it); q/k go to
                # DVE so exp never queues behind them at H starts.
                nc.scalar.activation(vT_s[:, sl], accs[0][:], Id,
                                     bias=bv_b)
                prek = prepool.tile([128, TQ], BF16, tag="prek")
                nc.vector.tensor_scalar(prek[:], accs[1][:], 1.0,
                                        bk_b, mybir.AluOpType.mult,
                                        mybir.AluOpType.add)
                pre0 = prepool.tile([128, TQ], BF16, tag="pre0")
                pre1 = prepool.tile([128, TQ], BF16, tag="pre1")
                nc.vector.tensor_scalar(pre0[:], accs[2][:], scale,
                                        bq0, mybir.AluOpType.mult,
                                        mybir.AluOpType.add)
                nc.vector.tensor_scalar(pre1[:], accs[3][:], scale,
                                        bq1, mybir.AluOpType.mult,
                                        mybir.AluOpType.add)
                rope(kr_s[:, sl], prek, t)
                rope(qr[0][:, sl], pre0, t)
                rope(qr[1][:, sl], pre1, t)

        def vtrans(t):
            # transpose t's 4 new v tiles to natural layout (fp32 PE path),
            # evict bf16 on ACT
            for tk in range(4 * t, 4 * t + 4):
                vtp = psum.tile([128, D], F32, tag="acc", bufs=4, name="vtp")
                nc.tensor.transpose(vtp[:], vT_s[:, bass.ts(tk, D)],
                                    ident_s[:])
                nc.vector.tensor_copy(v_s[:, bass.ts(tk, D)], vtp[:])

        def attn_heads(tq):
            ntk = 4 * tq + 4
            # Diagonal tiles first: their exp->mask chains complete while the
            # (mask-free) off-diagonal tiles stream, so the masked 128-column
            # matmuls (emitted after two off-diagonal consumes) never stall
            # the PE. For tq=0 every tile is diagonal; keep the plain order
            # with inline masked consumes there.
            if tq == 0:
                order = list(range(4))
            else:
                # one off-diagonal tile first: its full-width consume
                # initializes every PSUM column (start=True), so all later
                # partial-width writes accumulate
                order = ([0] + list(range(4 * tq, ntk))
                         + list(range(1, 4 * tq)))
            for h in range(HL):
                sump = psum.tile([128, TQ], F32, tag="nrm", bufs=2,
                                 name="sump")
                yp = psum.tile([128, TQ], F32, tag="nrm", bufs=2, name="yp")

                def score_exp(tk):
                    r = max(tk * D - tq * TQ, 0)  # masked col prefix
                    rr = min(r, TQ - 2 * D)       # keep free dim >= 256
                    sp_ = psum.tile([128, TQ], F32, tag="sp", bufs=2,
                                    name="sp")
                    nc.tensor.matmul(
                        sp_[:, rr:], kr_s[:, bass.ts(tk, D)],
                        qr[h][:, tq * TQ + rr:(tq + 1) * TQ],
                        start=True, stop=True)
                    pt = ppool.tile([128, TQ], BF16, tag="p")
                    nc.scalar.activation(pt[:, rr:], sp_[:, rr:], Exp)
                    if tk * D >= tq * TQ:  # diagonal: causal mask
                        if r > rr:
                            nc.gpsimd.tensor_mul(pt[:, rr:r + D],
                                                 pt[:, rr:r + D], tri2_s)
                        else:
                            nc.gpsimd.tensor_mul(pt[:, r:r + D],
                                                 pt[:, r:r + D], tri_s)
                    return pt, r, rr

                def consume(pt, lo, hi, tkv, first=False, last=False):
                    # exactly one start=True per accumulation group: it
                    # clears has_written for the whole bank, and its write
                    # must cover all columns later writes accumulate into
                    nc.tensor.matmul(sump[:, lo:hi], ones_s, pt[:, lo:hi],
                                     start=first, stop=last)
                    nc.tensor.matmul(yp[:, lo:hi], v_s[:, bass.ts(tkv, D)],
                                     pt[:, lo:hi], start=first, stop=last)

                pts = {}
                deferred = []
                ndone = 0
                for i in range(len(order) + 1):
                    if i < len(order):
                        pts[order[i]] = score_exp(order[i])
                    if i >= 1:
                        tk = order[i - 1]
                        pt, r, rr = pts.pop(tk)
                        if tq == 0:
                            # tile 0 consumed full-width inline (only its
                            # mask - the first ready - gates the PE); later
                            # tiles defer their masked columns to the end
                            if tk == 0:
                                consume(pt, 0, TQ, tk, first=True)
                                continue
                            deferred.append((pt, r, tk))
                            if r + D < TQ:
                                consume(pt, r + D, TQ, tk)
                            if tk == ntk - 1:
                                for j, (pt_, r_, tk_) in enumerate(deferred):
                                    consume(pt_, r_, r_ + D, tk_,
                                            last=(j == len(deferred) - 1))
                                deferred = []
                        elif tk * D >= tq * TQ:  # diagonal
                            # clean columns now; masked ones deferred past
                            # the exp->mask chain
                            deferred.append((pt, r, tk))
                            if r + D < TQ:
                                consume(pt, r + D, TQ, tk)
                        else:
                            ndone += 1
                            consume(pt, 0, TQ, tk, first=(ndone == 1),
                                    last=(ndone == 4 * tq))
                            if ndone == 3:
                                # flush after the 3rd off-diagonal consume:
                                # enough slack for the diagonal exp->mask
                                # chains without starving the pt pool
                                for pt_, r_, tk_ in deferred:
                                    consume(pt_, r_, r_ + D, tk_)
                                deferred = []
                rec = rpool.tile([128, TQ], F32, tag="rec")
                nc.vector.reciprocal_approx_fast(rec[:], sump[:])
                nc.vector.tensor_mul(ys[h][:, bass.ts(tq, TQ)], yp[:],
                                     rec[:])

        def o_proj(tq):
            # o_proj for the 4 row-tiles of block tq; evict the 4 column
            # tiles into one wide bf16 tile -> single DMA with 4KB lines.
            # Runs one slot after attn_heads(tq) so every input is ready.
            for tt in range(4):
                t = 4 * tq + tt
                wide = opool.tile([128, C], BF16, tag="oev")
                last_tile = tq == 3 and tt == 3
                for n in range(NT):
                    op_ = psum.tile([128, TQ], F32, tag="acc", bufs=4,
                                    name="op")
                    for h in range(HL):
                        nc.tensor.matmul(
                            op_[:], ys[h][:, bass.ts(t, D)],
                            wo_s[:, h * C + n * TQ:h * C + (n + 1) * TQ],
                            start=(h == 0), stop=(h == HL - 1))
                    if n == 3:
                        nc.scalar.copy(wide[:, bass.ts(n, TQ)], op_[:])
                    else:
                        nc.vector.tensor_copy(wide[:, bass.ts(n, TQ)], op_[:])
                    if last_tile:
                        # drain the final row-tile per column block on
                        # alternating queues so the tail is ~1 transfer deep
                        eng = nc.sync if n % 2 else nc.gpsimd
                        eng.dma_start(
                            out=out_d[bass.ts(t, D), n * TQ:(n + 1) * TQ],
                            in_=wide[:, bass.ts(n, TQ)])
                if not last_tile:
                    if tq >= 2 and tt % 2 == 1:
                        nc.sync.dma_start(out=out_d[bass.ts(t, D), :],
                                          in_=wide[:])
                    else:
                        nc.gpsimd.dma_start(out=out_d[bass.ts(t, D), :],
                                            in_=wide[:])

        proj_pair(0)
        vtrans(0)
        attn_heads(0)
        vtrans(1)
        attn_heads(1)
        o_proj(0)
        proj_pair(1)
        o_proj(1)
        vtrans(2)
        attn_heads(2)
        vtrans(3)
        attn_heads(3)
        o_proj(2)
        o_proj(3)
    nc.compile()
    return nc


def _get_nc():
    if "nc" not in _CACHE:
        _CACHE["nc"] = _build()
    return _CACHE["nc"]


def _prep_inputs(x, cos, sin, Wq, bq, Wk, bk, Wv, bv, Wo):
    f = np.float32
    bf = ml_dtypes.bfloat16
    xT = np.ascontiguousarray(x[0].T, dtype=bf)
    cosT = np.ascontiguousarray(cos[0].T, dtype=bf)
    sinT = np.asarray(sin[0].T, dtype=f)
    sins = np.concatenate([-sinT[:64], sinT[64:]], axis=0)
    sins = np.ascontiguousarray(sins, dtype=bf)
    idx = np.arange(D)
    tri = (idx[:, None] <= idx[None, :]).astype(bf)
    mcon = np.concatenate(
        [tri, np.zeros((D, D), bf), tri, np.ones((D, D), bf)], axis=1)
    mcon = np.ascontiguousarray(mcon)
    scale = np.float32(1.0 / math.sqrt(D))
    in_maps = []

    def ptile(a):
        """[K*128, N] -> partition-major [128, K*N] matching the SBUF tiles."""
        k = a.shape[0] // 128
        return np.ascontiguousarray(
            a.reshape(k, 128, a.shape[1]).transpose(1, 0, 2).reshape(128, -1)
            .astype(bf))

    for m in range(NCORES):
        g = m // 2
        wq_m = ptile(Wq[m * 256:(m + 1) * 256, :].T.astype(f))
        wk_m = ptile(Wk[g * 128:(g + 1) * 128, :].T.astype(f))
        wv_m = ptile(Wv[g * 128:(g + 1) * 128, :].T.astype(f))
        wo_m = ptile(Wo[:, m * 256:(m + 1) * 256].T.astype(f))
        bq_m = (bq[m * 256:(m + 1) * 256] * scale).reshape(HL, D).T
        bcon = np.stack([bq_m[:, 0], bq_m[:, 1],
                         bk[g * 128:(g + 1) * 128],
                         bv[g * 128:(g + 1) * 128]], axis=1)
        bcon = np.ascontiguousarray(bcon, dtype=f)
        in_maps.append({
            "xt": xT, "wq": wq_m, "wk": wk_m, "wv": wv_m, "wo": wo_m,
            "bcon": bcon, "mcon": mcon, "cost": cosT, "sins": sins,
        })
    return in_maps


def kernel(x, cos, sin, Wq, bq, Wk, bk, Wv, bv, Wo, _trace=False):
    x, cos, sin = np.asarray(x), np.asarray(cos), np.asarray(sin)
    Wq, bq = np.asarray(Wq), np.asarray(bq)
    Wk, bk = np.asarray(Wk), np.asarray(bk)
    Wv, bv = np.asarray(Wv), np.asarray(bv)
    Wo = np.asarray(Wo)
    nc = _get_nc()
    in_maps = _prep_inputs(x, cos, sin, Wq, bq, Wk, bk, Wv, bv, Wo)
    res = run_bass_kernel_spmd(nc, in_maps, core_ids=list(range(NCORES)),
                               trace=_trace)
    out = res.results[0]["out"].astype(np.float64)
    for m in range(1, NCORES):
        out += res.results[m]["out"]
    out = out.astype(np.float32).reshape(B, T, C)
    if _trace:
        _CACHE["last_result"] = res
    return out


# revision 32
# speedup vs baseline: 1.0110x; 1.0066x over previous
"""Trainium2 Bass kernel: causal self-attention (GQA + RoPE) for
B=1, T=2048, C=2048, H=16 query heads, HKV=4 KV heads, D=128.

Sharding: tensor-parallel over heads across 8 NeuronCores. Core m computes
query heads {2m, 2m+1} and the single KV head (m//2) those heads attend to,
plus the o_proj partial product for its 256 input columns. The host sums the
8 partial outputs (the TP all-reduce).

All matmul operands are bf16 (measured end-to-end emulation error 3.6e-3
against the fp64 reference, vs the 2e-2 gate); PSUM accumulation stays fp32.
bf16 halves HBM traffic (x alone is 8.4MB/core), halves DVE element cost,
and halves LDWEIGHTS time vs fp32r. The v-transpose chain stays fp32
(PE matmul-transpose; bf16 DMA-transpose would xbar-serialize the queues).

Schedule (program order = PE order). H(tq) = scores/softmax/AV for one query
block, O(tq) = its o_proj. O(tq) is deferred one slot so the DVE ymul that
produces ys always completes during unrelated PE work, and O's inputs are
never waited on:

  warmup MMs (beats the HAM clock-gate: PE busy from ~7us so the 2.4GHz
  un-throttle lands right as the first real matmul issues)
  pair0 [t0,t1]  vt0 H0 vt1 H1 O0  pair1 [t2,t3]  O1 vt2 H2 vt3 H3 O2 O3

Each projection half holds only 4 PSUM banks (one x chunk [128,1024] feeds
both halves back-to-back), so one unified PSUM pool (acc=4, sp=2, nrm=2
banks) serves the whole kernel with no pool-transition barriers.

DMA plan: first-use-ordered weight chunks then wo on the gpsimd SWDGE queue
(~2MB before the x stream peaks); x chunks + per-tile cos/sin slices + rope
partition-swaps on the sync queue; output rides gpsimd early, both queues
for the last two blocks to halve the tail drain. Engine plan: ACT does exp,
v-eviction and a quarter of o-evictions; DVE does q/k evictions, RoPE,
normalization and the rest; GpSimd does the causal-mask multiplies (SBUF
only) so they never queue behind the exp/eviction streams.

Attention per (tq, h): ST = k'T.T @ q'T per 128-key tile (causally skipped
column prefixes), P = exp(ST) on ACT (bf16 out, no max subtraction: logits
are bounded), diagonal masked by a triangular multiply, column sums via
ones-matmul + AV matmul accumulated in PSUM, y normalized by
reciprocal(sums), o_proj partials evicted bf16 and DMA'd.
"""

import math
import numpy as np
from contextlib import ExitStack

import ml_dtypes

import concourse.bass as bass
import concourse.bacc as bacc
import concourse.tile as tile
from concourse import mybir
from concourse.bass_utils import run_bass_kernel_spmd
from concourse.masks import make_identity

B, T, C = 1, 2048, 2048
H, HKV = 16, 4
D = 128
NCORES = 8
HL = H // NCORES          # query heads per core
TQ = 512                  # query tile width (one fp32 PSUM bank)
NT = T // TQ              # 4 query tiles
NK = T // D               # 16 key tiles
NCT = C // 128            # 16 contraction tiles over the model dim
F32 = mybir.dt.float32
BF16 = mybir.dt.bfloat16
Id = mybir.ActivationFunctionType.Identity
Exp = mybir.ActivationFunctionType.Exp

_CACHE: dict = {}


def _build():
    nc = bacc.Bacc(None, target_bir_lowering=False, debug=False)
    scale = 1.0 / math.sqrt(D)
    with tile.TileContext(nc) as tc, ExitStack() as ctx:
        dram = ctx.enter_context(tc.tile_pool(name="dram", bufs=1, space="DRAM"))

        def din(name, shape, dt=BF16):
            return dram.tile(shape, dt, kind="ExternalInput", name=name,
                             uniquify=False)

        xt_d = din("xt", [C, T])          # x[0].T
        # weights host-pre-tiled to SBUF layout [128, ...]; chunk c of wq at
        # cols c*HL*D.., of wk/wv at c*D..
        wq_d = din("wq", [128, NCT * HL * D])
        wk_d = din("wk", [128, NCT * D])
        wv_d = din("wv", [128, NCT * D])
        wo_d = din("wo", [128, HL * C])
        bcon_d = din("bcon", [D, 4], F32)   # [bq0 bq1 bk bv], bq pre-scaled
        mcon_d = din("mcon", [D, 4 * D])    # [tri | tri2(2D) | ones]
        cos_d = din("cost", [D, T])         # cos[0].T
        sin_d = din("sins", [D, T])         # sin[0].T with rows 0:64 negated
        out_d = dram.tile([T, C], BF16, kind="ExternalOutput",
                          name="out", uniquify=False)

        const = ctx.enter_context(tc.tile_pool(name="const", bufs=1))
        wq_s = const.tile([128, NCT * HL * D], BF16, name="wq_s")
        wk_s = const.tile([128, NCT * D], BF16, name="wk_s")
        wv_s = const.tile([128, NCT * D], BF16, name="wv_s")
        wo_s = const.tile([128, HL * C], BF16, name="wo_s")
        bcon_s = const.tile([128, 4], F32, name="bcon_s")
        mcon_s = const.tile([128, 4 * D], BF16, name="mcon_s")
        cos_s = const.tile([128, T], BF16, name="cos_s")
        sin_s = const.tile([128, T], BF16, name="sin_s")
        ident_s = const.tile([128, 128], F32, name="ident_s")
        warm_s = const.tile([128, 128], F32, name="warm_s")
        # identity first: it runs on the GpSimd ENGINE ahead of the weight
        # dma_start issues, so the PE warmup below can begin at ~7us.
        make_identity(nc, ident_s[:])
        # Weight loads on the GpSimd SWDGE queue in first-use order (proj
        # matmuls per c go v,k,q0,q1), chunked so the c=0 matmuls wait on
        # one ~128KB chunk rather than the full set. wo follows immediately
        # (first needed by O0 at ~1/4 of the kernel).
        WCH = 4                           # c-chunks per weight DMA
        for g in range(NCT // WCH):
            qsl = bass.ts(g, WCH * HL * D)
            ksl = bass.ts(g, WCH * D)
            nc.gpsimd.dma_start(out=wv_s[:, ksl], in_=wv_d[:, ksl])
            nc.gpsimd.dma_start(out=wk_s[:, ksl], in_=wk_d[:, ksl])
            nc.gpsimd.dma_start(out=wq_s[:, qsl], in_=wq_d[:, qsl])
        for g in range(2):
            osl = bass.ts(g, C)
            nc.gpsimd.dma_start(out=wo_s[:, osl], in_=wo_d[:, osl])

        bq0, bq1 = bcon_s[:, 0:1], bcon_s[:, 1:2]
        bk_b, bv_b = bcon_s[:, 2:3], bcon_s[:, 3:4]
        tri_s = mcon_s[:, 0:D]
        tri2_s = mcon_s[:, D:3 * D]
        ones_s = mcon_s[:, 3 * D:4 * D]

        act = ctx.enter_context(tc.tile_pool(name="act", bufs=1))
        qr = [act.tile([128, T], BF16, name=f"qr{h}_s") for h in range(HL)]
        kr_s = act.tile([128, T], BF16, name="kr_s")
        vT_s = act.tile([128, T], F32, name="vT_s")
        v_s = act.tile([128, NK * D], BF16, name="v_s")
        ys = [act.tile([128, T], BF16, name=f"y{h}_s") for h in range(HL)]

        xpool = ctx.enter_context(tc.tile_pool(name="xpool", bufs=NCT))
        prepool = ctx.enter_context(tc.tile_pool(name="prepool", bufs=4))
        rpool = ctx.enter_context(tc.tile_pool(name="rpool", bufs=3))
        ppool = ctx.enter_context(tc.tile_pool(name="ppool", bufs=8))
        opool = ctx.enter_context(tc.tile_pool(name="opool", bufs=3))
        # single PSUM pool for the whole kernel: acc(4) + sp(2) + nrm(2) = 8
        # banks; no pool-transition barriers between phases.
        psum = ctx.enter_context(tc.tile_pool(name="psum", bufs=1,
                                              space="PSUM"))

        # PE warmup: ~3.4us of accumulating matmuls on the identity so the
        # HAM clock-gate opens to 2.4GHz right as the first x chunk lands.
        # (Accumulation group so DCE can't drop the intermediate writes.)
        warm = psum.tile([128, 128], F32, tag="acc", bufs=4, name="warm")
        NWARM = 12
        for i in range(NWARM):
            nc.tensor.matmul(warm[:], ident_s[:], ident_s[:],
                             start=(i == 0), stop=(i == NWARM - 1))
        nc.scalar.copy(warm_s[:], warm[:])

        def rope(dst, pre, t):
            """dst = pre*cos + rot_half(pre)*sin on columns [t*TQ,(t+1)*TQ).

            All bf16 on DVE (2x 16-bit mode). Partition-swap copies ride the
            sync queue behind this pair's x chunks.
            """
            sl = bass.ts(t, TQ)
            rot = rpool.tile([128, TQ], BF16, tag="rot")
            nc.sync.dma_start(out=rot[0:64, :], in_=pre[64:128, :])
            nc.sync.dma_start(out=rot[64:128, :], in_=pre[0:64, :])
            nc.vector.tensor_mul(rot[:], rot[:], sin_s[:, sl])
            tmp = rpool.tile([128, TQ], BF16, tag="rtmp")
            nc.vector.tensor_mul(tmp[:], pre[:], cos_s[:, sl])
            nc.vector.tensor_add(dst, tmp[:], rot[:])

        def proj_pair(tp):
            # One x chunk [128, 1024] per c feeds both halves (one dma_start
            # each: the ~0.65us per-issue sequencer cost makes finer chunks
            # a net loss). Each half keeps its own 4-bank accumulator set
            # processed back-to-back so proj never holds more than 4 PSUM
            # banks (attn tags keep theirs).
            xts = []
            for c in range(NCT):
                xt = xpool.tile([128, 2 * TQ], BF16, tag="x")
                nc.sync.dma_start(
                    out=xt[:],
                    in_=xt_d[bass.ts(c, 128), tp * 2 * TQ:(tp + 1) * 2 * TQ])
                xts.append(xt)
            # cos/sin slices for this pair's two tiles, behind the x chunks
            for t in (2 * tp, 2 * tp + 1):
                sl = bass.ts(t, TQ)
                nc.sync.dma_start(out=cos_s[:, sl], in_=cos_d[:, sl])
                nc.sync.dma_start(out=sin_s[:, sl], in_=sin_d[:, sl])
            if tp == 0:
                # biases/masks behind pair0's x so the x stream (which paces
                # the first c-loop) owns the early bandwidth; these are only
                # needed at eviction (~24us) and H0 (~32us).
                nc.sync.dma_start(out=bcon_s[:], in_=bcon_d[:])
                nc.sync.dma_start(out=mcon_s[:], in_=mcon_d[:])
            for half in range(2):
                t = 2 * tp + half
                sl = bass.ts(t, TQ)
                # issue order (v,k,q0,q1) matches eviction order below so the
                # next half's first MM waits only on the first eviction
                accs = [psum.tile([128, TQ], F32, tag="acc", bufs=4,
                                  name=f"acc{j}") for j in range(4)]
                for c in range(NCT):
                    xh = xts[c][:, bass.ts(half, TQ)]
                    st, sp = (c == 0), (c == NCT - 1)
                    base = c * HL * D
                    nc.tensor.matmul(accs[0][:], wv_s[:, bass.ts(c, D)],
                                     xh, start=st, stop=sp)
                    nc.tensor.matmul(accs[1][:], wk_s[:, bass.ts(c, D)],
                                     xh, start=st, stop=sp)
                    nc.tensor.matmul(accs[2][:], wq_s[:, base:base + D],
                                     xh, start=st, stop=sp)
                    nc.tensor.matmul(accs[3][:],
                                     wq_s[:, base + D:base + 2 * D],
                                     xh, start=st, stop=sp)
                # ACT keeps only the v eviction (vtrans needs it); q/k go to
                # DVE so exp never queues behind them at H starts.
                nc.scalar.activation(vT_s[:, sl], accs[0][:], Id,
                                     bias=bv_b)
                prek = prepool.tile([128, TQ], BF16, tag="prek")
                nc.vector.tensor_scalar(prek[:], accs[1][:], 1.0,
                                        bk_b, mybir.AluOpType.mult,
                                        mybir.AluOpType.add)
                pre0 = prepool.tile([128, TQ], BF16, tag="pre0")
                pre1 = prepool.tile([128, TQ], BF16, tag="pre1")
                nc.vector.tensor_scalar(pre0[:], accs[2][:], scale,
                                        bq0, mybir.AluOpType.mult,
                                        mybir.AluOpType.add)
                nc.vector.tensor_scalar(pre1[:], accs[3][:], scale,
                                        bq1, mybir.AluOpType.mult,
                                        mybir.AluOpType.add)
                rope(kr_s[:, sl], prek, t)
                rope(qr[0][:, sl], pre0, t)
                rope(qr[1][:, sl], pre1, t)

        def vtrans(t):
            # transpose t's 4 new v tiles to natural layout (fp32 PE path),
            # evict bf16 on ACT
            for tk in range(4 * t, 4 * t + 4):
                vtp = psum.tile([128, D], F32, tag="acc", bufs=4, name="vtp")
                nc.tensor.transpose(vtp[:], vT_s[:, bass.ts(tk, D)],
                                    ident_s[:])
                nc.vector.tensor_copy(v_s[:, bass.ts(tk, D)], vtp[:])

        def attn_heads(tq):
            ntk = 4 * tq + 4
            # Diagonal tiles first: their exp->mask chains complete while the
            # (mask-free) off-diagonal tiles stream, so the masked 128-column
            # matmuls (emitted after two off-diagonal consumes) never stall
            # the PE. For tq=0 every tile is diagonal; keep the plain order
            # with inline masked consumes there.
            if tq == 0:
                order = list(range(4))
            else:
                # one off-diagonal tile first: its full-width consume
                # initializes every PSUM column (start=True), so all later
                # partial-width writes accumulate
                order = ([0] + list(range(4 * tq, ntk))
                         + list(range(1, 4 * tq)))
            for h in range(HL):
                sump = psum.tile([128, TQ], F32, tag="nrm", bufs=2,
                                 name="sump")
                yp = psum.tile([128, TQ], F32, tag="nrm", bufs=2, name="yp")

                def score_exp(tk):
                    r = max(tk * D - tq * TQ, 0)  # masked col prefix
                    rr = min(r, TQ - 2 * D)       # keep free dim >= 256
                    sp_ = psum.tile([128, TQ], F32, tag="sp", bufs=2,
                                    name="sp")
                    nc.tensor.matmul(
                        sp_[:, rr:], kr_s[:, bass.ts(tk, D)],
                        qr[h][:, tq * TQ + rr:(tq + 1) * TQ],
                        start=True, stop=True)
                    pt = ppool.tile([128, TQ], BF16, tag="p")
                    nc.scalar.activation(pt[:, rr:], sp_[:, rr:], Exp)
                    if tk * D >= tq * TQ:  # diagonal: causal mask
                        # DVE for H0/H1: it is idle there (o-evictions only
                        # start with O0) and ~3x faster than GpSimd on
                        # 2-input ops, shortening the exp->mask->consume
                        # chain; H2/H3 keep GpSimd so masks never queue
                        # behind the previous O block's DVE evictions
                        meng = nc.vector if tq <= 1 else nc.gpsimd
                        if r > rr:
                            meng.tensor_mul(pt[:, rr:r + D],
                                            pt[:, rr:r + D], tri2_s)
                        else:
                            meng.tensor_mul(pt[:, r:r + D],
                                            pt[:, r:r + D], tri_s)
                    return pt, r, rr

                def consume(pt, lo, hi, tkv, first=False, last=False):
                    # exactly one start=True per accumulation group: it
                    # clears has_written for the whole bank, and its write
                    # must cover all columns later writes accumulate into
                    nc.tensor.matmul(sump[:, lo:hi], ones_s, pt[:, lo:hi],
                                     start=first, stop=last)
                    nc.tensor.matmul(yp[:, lo:hi], v_s[:, bass.ts(tkv, D)],
                                     pt[:, lo:hi], start=first, stop=last)

                pts = {}
                deferred = []
                ndone = 0
                for i in range(len(order) + 1):
                    if i < len(order):
                        pts[order[i]] = score_exp(order[i])
                    if i >= 1:
                        tk = order[i - 1]
                        pt, r, rr = pts.pop(tk)
                        if tq == 0:
                            # tile 0 consumed full-width inline (only its
                            # mask - the first ready - gates the PE); later
                            # tiles defer their masked columns to the end
                            if tk == 0:
                                consume(pt, 0, TQ, tk, first=True)
                                continue
                            deferred.append((pt, r, tk))
                            if r + D < TQ:
                                consume(pt, r + D, TQ, tk)
                            if tk == ntk - 1:
                                for j, (pt_, r_, tk_) in enumerate(deferred):
                                    consume(pt_, r_, r_ + D, tk_,
                                            last=(j == len(deferred) - 1))
                                deferred = []
                        elif tk * D >= tq * TQ:  # diagonal
                            # clean columns now; masked ones deferred past
                            # the exp->mask chain
                            deferred.append((pt, r, tk))
                            if r + D < TQ:
                                consume(pt, r + D, TQ, tk)
                        else:
                            ndone += 1
                            consume(pt, 0, TQ, tk, first=(ndone == 1),
                                    last=(ndone == 4 * tq))
                            if ndone == 3:
                                # flush after the 3rd off-diagonal consume:
                                # enough slack for the diagonal exp->mask
                                # chains without starving the pt pool
                                for pt_, r_, tk_ in deferred:
                                    consume(pt_, r_, r_ + D, tk_)
                                deferred = []
                rec = rpool.tile([128, TQ], F32, tag="rec")
                nc.vector.reciprocal_approx_fast(rec[:], sump[:])
                nc.vector.tensor_mul(ys[h][:, bass.ts(tq, TQ)], yp[:],
                                     rec[:])

        def o_proj(tq):
            # o_proj for the 4 row-tiles of block tq; evict the 4 column
            # tiles into one wide bf16 tile -> single DMA with 4KB lines.
            # Runs one slot after attn_heads(tq) so every input is ready.
            for tt in range(4):
                t = 4 * tq + tt
                wide = opool.tile([128, C], BF16, tag="oev")
                last_tile = tq == 3 and tt == 3
                for n in range(NT):
                    op_ = psum.tile([128, TQ], F32, tag="acc", bufs=4,
                                    name="op")
                    for h in range(HL):
                        nc.tensor.matmul(
                            op_[:], ys[h][:, bass.ts(t, D)],
                            wo_s[:, h * C + n * TQ:h * C + (n + 1) * TQ],
                            start=(h == 0), stop=(h == HL - 1))
                    if n == 3:
                        nc.scalar.copy(wide[:, bass.ts(n, TQ)], op_[:])
                    else:
                        nc.vector.tensor_copy(wide[:, bass.ts(n, TQ)], op_[:])
                    if last_tile:
                        # drain the final row-tile per column block on
                        # alternating queues so the tail is ~1 transfer deep
                        eng = nc.sync if n % 2 else nc.gpsimd
                        eng.dma_start(
                            out=out_d[bass.ts(t, D), n * TQ:(n + 1) * TQ],
                            in_=wide[:, bass.ts(n, TQ)])
                if not last_tile:
                    if tq >= 2 and tt % 2 == 1:
                        nc.sync.dma_start(out=out_d[bass.ts(t, D), :],
                                          in_=wide[:])
                    else:
                        nc.gpsimd.dma_start(out=out_d[bass.ts(t, D), :],
                                            in_=wide[:])

        proj_pair(0)
        vtrans(0)
        attn_heads(0)
        vtrans(1)
        attn_heads(1)
        o_proj(0)
        proj_pair(1)
        o_proj(1)
        vtrans(2)
        attn_heads(2)
        vtrans(3)
        attn_heads(3)
        o_proj(2)
        o_proj(3)
    nc.compile()
    return nc


def _get_nc():
    if "nc" not in _CACHE:
        _CACHE["nc"] = _build()
    return _CACHE["nc"]


def _prep_inputs(x, cos, sin, Wq, bq, Wk, bk, Wv, bv, Wo):
    f = np.float32
    bf = ml_dtypes.bfloat16
    xT = np.ascontiguousarray(x[0].T, dtype=bf)
    cosT = np.ascontiguousarray(cos[0].T, dtype=bf)
    sinT = np.asarray(sin[0].T, dtype=f)
    sins = np.concatenate([-sinT[:64], sinT[64:]], axis=0)
    sins = np.ascontiguousarray(sins, dtype=bf)
    idx = np.arange(D)
    tri = (idx[:, None] <= idx[None, :]).astype(bf)
    mcon = np.concatenate(
        [tri, np.zeros((D, D), bf), tri, np.ones((D, D), bf)], axis=1)
    mcon = np.ascontiguousarray(mcon)
    scale = np.float32(1.0 / math.sqrt(D))
    in_maps = []

    def ptile(a):
        """[K*128, N] -> partition-major [128, K*N] matching the SBUF tiles."""
        k = a.shape[0] // 128
        return np.ascontiguousarray(
            a.reshape(k, 128, a.shape[1]).transpose(1, 0, 2).reshape(128, -1)
            .astype(bf))

    for m in range(NCORES):
        g = m // 2
        wq_m = ptile(Wq[m * 256:(m + 1) * 256, :].T.astype(f))
        wk_m = ptile(Wk[g * 128:(g + 1) * 128, :].T.astype(f))
        wv_m = ptile(Wv[g * 128:(g + 1) * 128, :].T.astype(f))
        wo_m = ptile(Wo[:, m * 256:(m + 1) * 256].T.astype(f))
        bq_m = (bq[m * 256:(m + 1) * 256] * scale).reshape(HL, D).T
        bcon = np.stack([bq_m[:, 0], bq_m[:, 1],
                         bk[g * 128:(g + 1) * 128],
                         bv[g * 128:(g + 1) * 128]], axis=1)
        bcon = np.ascontiguousarray(bcon, dtype=f)
        in_maps.append({
            "xt": xT, "wq": wq_m, "wk": wk_m, "wv": wv_m, "wo": wo_m,
            "bcon": bcon, "mcon": mcon, "cost": cosT, "sins": sins,
        })
    return in_maps


def kernel(x, cos, sin, Wq, bq, Wk, bk, Wv, bv, Wo, _trace=False):
    x, cos, sin = np.asarray(x), np.asarray(cos), np.asarray(sin)
    Wq, bq = np.asarray(Wq), np.asarray(bq)
    Wk, bk = np.asarray(Wk), np.asarray(bk)
    Wv, bv = np.asarray(Wv), np.asarray(bv)
    Wo = np.asarray(Wo)
    nc = _get_nc()
    in_maps = _prep_inputs(x, cos, sin, Wq, bq, Wk, bk, Wv, bv, Wo)
    res = run_bass_kernel_spmd(nc, in_maps, core_ids=list(range(NCORES)),
                               trace=_trace)
    out = res.results[0]["out"].astype(np.float64)
    for m in range(1, NCORES):
        out += res.results[m]["out"]
    out = out.astype(np.float32).reshape(B, T, C)
    if _trace:
        _CACHE["last_result"] = res
    return out
